# revision 1
# baseline (speedup 1.0000x reference)
"""DeeperGCN (GENConv softmax-aggr, 3 layers) on 8 TRN2 NeuronCores.

Sharding: nodes partitioned contiguously across cores (PC/core, zero-padded
to NC*PC total).  Edges partitioned by dst node range, sorted by dst, grouped
into W-node blocks padded to a fixed number of 128-edge subtiles.  Per-layer
halo exchange of node features via AllGather (bf16).  Softmax aggregation is
computed without max-subtraction (mathematically identical here since message
values are bounded): P=sum(exp), Q=sum(msg*exp) via per-subtile one-hot
selector matmuls accumulated in PSUM; agg = Q/(P+1e-16).
"""

import sys, math

sys.path.insert(0, "/opt/trn_rl_repo")

import numpy as np
import concourse.bass as bass
import concourse.bacc as bacc
import concourse.mybir as mybir
import concourse.tile as tile
from concourse.tile import TileContext
from concourse.masks import make_identity

F32 = mybir.dt.float32
BF16 = mybir.dt.bfloat16
I32 = mybir.dt.int32
AF = mybir.ActivationFunctionType
OP = mybir.AluOpType
AX = mybir.AxisListType

P = 128


def _round_up(a, b):
    return (a + b - 1) // b * b


def _bap(ap, extra, off_elems=0):
    """AP with extra trailing dims and an element offset."""
    return bass.AP(tensor=ap.tensor, offset=ap.offset + off_elems,
                   ap=list(ap.ap) + list(extra))


class Plan:
    """Compile-time constants + per-core host-prepped arrays."""

    def __init__(self, inputs, ncores=8, w_blk=64):
        x = np.asarray(inputs["x"], np.float32)
        edge_attr = np.asarray(inputs["edge_attr"], np.float32)
        ei = np.asarray(inputs["edge_index"])
        self.N, self.FIN = x.shape
        self.E, self.FE = edge_attr.shape
        self.H = int(np.asarray(inputs["enc_w"]).shape[1])
        self.L = int(np.asarray(inputs["mlp_w1"]).shape[0])
        self.H2 = int(np.asarray(inputs["mlp_w1"]).shape[2])
        self.COUT = int(np.asarray(inputs["lin_w"]).shape[1])
        self.NC = ncores
        self.W = w_blk
        assert self.H == 64 and self.FIN == 64 and self.H2 == 128 and self.FE == 16

        self.PC = _round_up(int(math.ceil(self.N / ncores)), P)  # nodes/core
        self.NPAD = self.PC * ncores
        self.NT = self.PC // P                    # node tiles per core
        self.NB = self.PC // self.W               # blocks per core
        for ggb in (7, 4, 2, 1):
            if self.NB % (2 * ggb) == 0:
                self.GGB = ggb
                break
        self.NGG = self.NB // self.GGB            # gather groups per core
        self.NMEGA = self.NGG // 2                # psum megas (2 GGs each)

        src = ei[0].astype(np.int64)
        dst = ei[1].astype(np.int64)
        order = np.argsort(dst, kind="stable")
        src_s = src[order].astype(np.int32)
        dst_s = dst[order].astype(np.int32)
        ea_s = edge_attr[order]

        blk = dst_s // self.W                      # global block id
        nblk_tot = self.NPAD // self.W
        counts = np.bincount(blk, minlength=nblk_tot)
        self.KS = max(2, int(math.ceil(counts.max() / P)))  # subtiles/block
        self.CAP = self.KS * P
        self.TS = self.NB * self.KS                # subtiles per core
        self.EP = self.TS * P                      # padded edge slots / core

        core_of_blk = (blk * self.W) // self.PC
        blk_local = blk - core_of_blk * self.NB
        pos_in_blk = np.arange(len(dst_s)) - np.concatenate(
            ([0], np.cumsum(counts)))[blk]
        slot_local = blk_local * self.CAP + pos_in_blk

        self.src_idx = np.zeros((ncores, P, self.TS), np.int32)
        self.eaT = np.zeros((ncores, 17, self.TS * P), np.float32)
        self.sel = np.zeros((ncores, P, self.TS * self.W), np.float32)
        self.eaT[:, 16, :] = 1.0

        rel = dst_s - blk * self.W
        s_sub = slot_local // P
        p_sub = slot_local % P
        c_arr = core_of_blk
        self.src_idx[c_arr, p_sub, s_sub] = src_s
        for k in range(16):
            self.eaT[c_arr, k, s_sub * P + p_sub] = ea_s[:, k]
        self.sel[c_arr, p_sub, s_sub * self.W + rel] = 1.0

        xpad = np.zeros((self.NPAD, self.FIN), np.float32)
        xpad[: self.N] = x
        self.x_t = np.ones((ncores, 65, self.PC), np.float32)
        for c in range(ncores):
            self.x_t[c, :64, :] = xpad[c * self.PC:(c + 1) * self.PC].T

        self.enc_w = np.concatenate(
            [np.asarray(inputs["enc_w"], np.float32),
             np.asarray(inputs["enc_b"], np.float32)[None, :]], 0)
        self.eenc_w = np.concatenate(
            [np.asarray(inputs["eenc_w"], np.float32),
             np.asarray(inputs["eenc_b"], np.float32)[None, :]], 0)
        w1 = np.asarray(inputs["mlp_w1"], np.float32)
        b1 = np.asarray(inputs["mlp_b1"], np.float32)
        self.w1 = np.concatenate([w1, b1[:, None, :]], 1)       # [L,65,128]
        self.w2 = np.asarray(inputs["mlp_w2"], np.float32)      # [L,128,64]
        self.b2 = np.asarray(inputs["mlp_b2"], np.float32)      # [L,64]
        self.mlp_ln_g = np.asarray(inputs["mlp_ln_g"], np.float32)
        self.mlp_ln_b = np.asarray(inputs["mlp_ln_b"], np.float32)
        self.ln_g = np.asarray(inputs["ln_g"], np.float32)
        self.ln_b = np.asarray(inputs["ln_b"], np.float32)
        self.t = np.asarray(inputs["t"], np.float32)
        self.lin_w = np.concatenate(
            [np.asarray(inputs["lin_w"], np.float32),
             np.asarray(inputs["lin_b"], np.float32)[None, :]], 0)
        assert np.all(self.t > 0)

        self.mlp_ln_id = [bool(np.all(self.mlp_ln_g[l] == 1.0)
                               and np.all(self.mlp_ln_b[l] == 0.0))
                          for l in range(self.L)]
        self.ln_id = [bool(np.all(self.ln_g[l] == 1.0)
                           and np.all(self.ln_b[l] == 0.0))
                      for l in range(self.L)]
        self.b2_zero = [bool(np.all(self.b2[l] == 0.0)) for l in range(self.L)]
        self.any_gb = not (all(self.mlp_ln_id) and all(self.ln_id))
        self.LN_EPS = 1e-5


def build(plan: "Plan", use_cce=True):
    pl = plan
    NC, PC, NT, NB, W, KS, TS, GGB = (
        pl.NC, pl.PC, pl.NT, pl.NB, pl.W, pl.KS, pl.TS, pl.GGB)
    SUBS_GG = GGB * KS
    L = pl.L
    COUT = pl.COUT

    nc = bacc.Bacc("TRN2", num_devices=NC)

    d_xt = nc.dram_tensor("x_t", (65, PC), F32, kind="ExternalInput")
    d_src = nc.dram_tensor("src_idx", (P, TS), I32, kind="ExternalInput")
    d_eaT = nc.dram_tensor("eaT", (17, TS * P), BF16, kind="ExternalInput")
    d_sel = nc.dram_tensor("sel", (P, TS * W), BF16, kind="ExternalInput")
    d_encw = nc.dram_tensor("enc_w", (65, 64), F32, kind="ExternalInput")
    d_eencw = nc.dram_tensor("eenc_w", (17, 64), BF16, kind="ExternalInput")
    d_w1 = nc.dram_tensor("w1", (L, 65, 128), F32, kind="ExternalInput")
    d_w2 = nc.dram_tensor("w2", (L, 128, 64), BF16, kind="ExternalInput")
    d_b2 = nc.dram_tensor("b2", (L, 64), F32, kind="ExternalInput")
    d_linw = nc.dram_tensor("lin_w", (65, COUT), F32, kind="ExternalInput")
    d_gb = None
    if pl.any_gb:
        d_gb = nc.dram_tensor("ln_gb", (2 * L, 192), F32, kind="ExternalInput")
    d_out = nc.dram_tensor("y_out", (PC, COUT), F32, kind="ExternalOutput")

    with TileContext(nc) as tc:
        import contextlib
        with contextlib.ExitStack() as stk:
            dram = stk.enter_context(
                tc.tile_pool(name="dram", bufs=1, space="DRAM"))
            const = stk.enter_context(tc.tile_pool(name="const", bufs=1))
            resid = stk.enter_context(tc.tile_pool(name="resid", bufs=1))

            cc_in = [dram.tile([PC, 64], BF16, name=f"cc_in{l}")
                     for l in range(L)]
            g_full = [dram.tile([NC * PC, 64], BF16, name=f"g_full{l}")
                      for l in range(L)]

            ident_f = const.tile([P, P], F32)
            make_identity(nc, ident_f[:])
            ident_b = const.tile([P, P], BF16)
            nc.vector.tensor_copy(ident_b[:], ident_f[:])
            src_sb = const.tile([P, TS], I32)
            nc.sync.dma_start(out=src_sb[:], in_=d_src[:])
            eenc_sb = const.tile([17, 64], BF16)
            nc.sync.dma_start(out=eenc_sb[:], in_=d_eencw[:])
            encw_sb = const.tile([65, 64], F32)
            nc.sync.dma_start(out=encw_sb[:], in_=d_encw[:])
            w1_sb = const.tile([65, L * 128], F32)
            nc.sync.dma_start(
                out=w1_sb[:].rearrange("p (l n) -> p l n", l=L),
                in_=d_w1.rearrange("l p n -> p l n"))
            w2_sb = const.tile([P, L * 64], BF16)
            nc.sync.dma_start(
                out=w2_sb[:].rearrange("p (l n) -> p l n", l=L),
                in_=d_w2.rearrange("l p n -> p l n"))
            b2_sb = const.tile([P, L * 64], F32)
            nc.gpsimd.dma_start(
                out=b2_sb[:].rearrange("p (l n) -> p l n", l=L),
                in_=bass.AP(tensor=d_b2.ap().tensor, offset=0,
                            ap=[[0, P], [64, L], [1, 64]]))
            linw_sb = const.tile([65, COUT], F32)
            nc.sync.dma_start(out=linw_sb[:], in_=d_linw[:])
            gb_sb = None
            if pl.any_gb:
                gb_sb = const.tile([P, 2 * L * 192], F32)
                nc.gpsimd.dma_start(
                    out=gb_sb[:].rearrange("p (l n) -> p l n", l=2 * L),
                    in_=bass.AP(tensor=d_gb.ap().tensor, offset=0,
                                ap=[[0, P], [192, 2 * L], [1, 192]]))

            eps_sb = const.tile([P, 1], F32)
            nc.vector.memset(eps_sb[:], 1e-5)
            h_res = resid.tile([P, NT * 64], F32)
            v_nm = resid.tile([P, NT * 64], F32)
            agg_nm = resid.tile([P, NT * 64], F32)
            h1_fm = resid.tile([65, NT * P], F32)
            nc.vector.memset(h1_fm[64:65, :], 1.0)
            z2_nm = resid.tile([P, NT * 128], BF16)
            z2_fm = resid.tile([P, NT * P], BF16)
            sq_sb = resid.tile([P, max(NT * 64, 512)], BF16)
            ztmp = resid.tile([P, 512 + max(NT * 64, 512)], F32)
            zbf = resid.tile([P, NT * 64], BF16)

            # ------------------------------------------------ phases
            def enc_phase():
                with tc.tile_pool(name="encp", bufs=1) as encp, \
                     tc.tile_pool(name="encps", bufs=2, space="PSUM") as encps:
                    xt_sb = encp.tile([65, PC], F32, tag="xt")
                    nc.sync.dma_start(out=xt_sb[:], in_=d_xt[:])
                    for ch in range((NT + 7) // 8):
                        t0, t1 = ch * 8, min(ch * 8 + 8, NT)
                        ps = encps.tile([P, 512], F32, tag="encps")
                        for t in range(t0, t1):
                            nc.tensor.matmul(
                                ps[:, (t - t0) * 64:(t - t0 + 1) * 64],
                                lhsT=xt_sb[:, t * P:(t + 1) * P],
                                rhs=encw_sb[:], start=True, stop=True)
                        nc.scalar.copy(out=v_nm[:, t0 * 64:t1 * 64],
                                       in_=ps[:, :(t1 - t0) * 64])
                        nc.vector.tensor_copy(out=zbf[:, t0 * 64:t1 * 64],
                                              in_=ps[:, :(t1 - t0) * 64])
                nc.sync.dma_start(
                    out=cc_in[0][:].rearrange("(t p) d -> p t d", p=P),
                    in_=zbf[:].rearrange("p (t d) -> p t d", d=64))
                nc.gpsimd.collective_compute(
                    "AllGather", OP.bypass,
                    ins=[cc_in[0][:]], outs=[g_full[0][:]],
                    replica_groups=[list(range(NC))])

            def edge_phase(l):
                tl = float(pl.t[l])
                wdt = BF16 if use_cce else F32
                with tc.tile_pool(name="ew", bufs=2) as ewp, \
                     tc.tile_pool(name="esel", bufs=3) as eselp, \
                     tc.tile_pool(name="eem", bufs=2) as eemp, \
                     tc.tile_pool(name="eeat", bufs=3) as eeatp, \
                     tc.tile_pool(name="eaps", bufs=3, space="PSUM") as eapsp, \
                     tc.tile_pool(name="megap", bufs=1, space="PSUM") as megap:
                    for m in range(pl.NMEGA):
                        mega = megap.tile([P, GGB * 128], F32, tag="mega")
                        for h in range(2):
                            gg = 2 * m + h
                            s0 = gg * SUBS_GG
                            w_t = ewp.tile([P, SUBS_GG * 64], wdt, tag="w")
                            sel_t = eselp.tile([P, SUBS_GG * W], BF16,
                                               tag="sel")
                            nc.sync.dma_start(
                                out=sel_t[:],
                                in_=d_sel[:, s0 * W:(s0 + SUBS_GG) * W])
                            em = eemp.tile([P, SUBS_GG * 128], BF16, tag="em")
                            eaps_list = []
                            for c0 in range(0, SUBS_GG, 8):
                                c1 = min(c0 + 8, SUBS_GG)
                                eat = eeatp.tile([17, 8 * P], BF16, tag="eaT")
                                nc.sync.dma_start(
                                    out=eat[:, :(c1 - c0) * P],
                                    in_=d_eaT[:, (s0 + c0) * P:(s0 + c1) * P])
                                eap = eapsp.tile([P, 512], F32, tag="eaps")
                                for s in range(c0, c1):
                                    nc.tensor.matmul(
                                        eap[:, (s - c0) * 64:(s - c0 + 1) * 64],
                                        lhsT=eat[:, (s - c0) * P:(s - c0 + 1) * P],
                                        rhs=eenc_sb[:], start=True, stop=True)
                                if use_cce:
                                    nc.scalar.copy(
                                        out=w_t[:, c0 * 64:c1 * 64],
                                        in_=eap[:, :(c1 - c0) * 64])
                                else:
                                    eaps_list.append((c0, c1, eap))
                            for s in range(SUBS_GG):
                                nc.gpsimd.indirect_dma_start(
                                    out=w_t[:, s * 64:(s + 1) * 64],
                                    out_offset=None,
                                    in_=g_full[l][:],
                                    in_offset=bass.IndirectOffsetOnAxis(
                                        ap=src_sb[:, s0 + s:s0 + s + 1],
                                        axis=0),
                                    compute_op=(OP.add if use_cce
                                                else OP.bypass))
                            em3 = em[:].rearrange("p (s d) -> p s d", d=128)
                            w3 = w_t[:].rearrange("p (s d) -> p s d", d=64)
                            if use_cce:
                                nc.vector.tensor_scalar(
                                    out=em3[:, :, 64:128], in0=w3,
                                    scalar1=0.0, scalar2=None, op0=OP.max)
                            else:
                                for c0, c1, eap in eaps_list:
                                    nc.vector.scalar_tensor_tensor(
                                        out=em3[:, c0:c1, 64:128],
                                        in0=w3[:, c0:c1, :], scalar=0.0,
                                        in1=eap[:, :(c1 - c0) * 64].rearrange(
                                            "p (s d) -> p s d", d=64),
                                        op0=OP.bypass, op1=OP.add)
                                nc.vector.tensor_scalar(
                                    out=em3[:, :, 64:128],
                                    in0=em3[:, :, 64:128],
                                    scalar1=0.0, scalar2=None, op0=OP.max)
                            nc.scalar.activation(
                                out=em3[:, :, 0:64], in_=em3[:, :, 64:128],
                                func=AF.Exp, scale=tl)
                            nc.vector.tensor_tensor(
                                out=em3[:, :, 64:128], in0=em3[:, :, 64:128],
                                in1=em3[:, :, 0:64], op=OP.mult)
                            for s in range(SUBS_GG):
                                sg = s0 + s
                                b_loc = sg // KS - 2 * m * GGB
                                s_in_b = sg % KS
                                pair, poff = b_loc // 2, (b_loc % 2) * 64
                                nc.tensor.matmul(
                                    mega[poff:poff + 64,
                                         pair * 128:(pair + 1) * 128],
                                    lhsT=sel_t[:, s * W:(s + 1) * W],
                                    rhs=em[:, s * 128:(s + 1) * 128],
                                    start=(s_in_b == 0),
                                    stop=(s_in_b == KS - 1))
                        mm = mega[:].rearrange("p (q two d) -> p q two d",
                                               two=2, d=64)
                        ptmp = ztmp[:, 0:GGB * 64].rearrange(
                            "p (q d) -> p q d", d=64)
                        nc.vector.tensor_scalar(
                            out=ptmp, in0=mm[:, :, 0, :],
                            scalar1=1e-16, scalar2=None, op0=OP.add)
                        nc.vector.reciprocal(out=ptmp, in_=ptmp)
                        nc.vector.tensor_tensor(
                            out=agg_nm[:, m * GGB * 64:(m + 1) * GGB * 64]
                                .rearrange("p (q d) -> p q d", d=64),
                            in0=mm[:, :, 1, :], in1=ptmp, op=OP.mult)

            def ln_relu_chunk(l, zps, nt, t0):
                d = 128
                z3 = zps[:, :nt * d].rearrange("p (t d) -> p t d", d=d)
                r1 = ztmp[:, 0:nt]
                nc.vector.tensor_reduce(out=r1, in_=z3, axis=AX.X, op=OP.add)
                nc.scalar.activation(out=sq_sb[:, 0:nt * d],
                                     in_=zps[:, :nt * d], func=AF.Square)
                r2 = ztmp[:, 8:8 + nt]
                nc.vector.tensor_reduce(
                    out=r2,
                    in_=sq_sb[:, 0:nt * d].rearrange("p (t d) -> p t d", d=d),
                    axis=AX.X, op=OP.add)
                mu = ztmp[:, 16:16 + nt]
                nc.vector.tensor_scalar(out=mu, in0=r1, scalar1=1.0 / d,
                                        scalar2=None, op0=OP.mult)
                m2 = ztmp[:, 24:24 + nt]
                nc.vector.tensor_tensor(out=m2, in0=mu, in1=mu, op=OP.mult)
                var = ztmp[:, 32:32 + nt]
                nc.vector.scalar_tensor_tensor(
                    out=var, in0=r2, scalar=1.0 / d, in1=m2,
                    op0=OP.mult, op1=OP.subtract)
                sd = ztmp[:, 40:40 + nt]
                nc.scalar.activation(out=sd, in_=var, func=AF.Sqrt,
                                     bias=eps_sb[:])
                rstd = ztmp[:, 48:48 + nt]
                nc.vector.reciprocal(out=rstd, in_=sd)
                ytmp = ztmp[:, 512:512 + nt * d].rearrange(
                    "p (t d) -> p t d", d=d)
                nc.vector.tensor_tensor(out=ytmp, in0=z3,
                                        in1=_bap(mu, [[0, d]]), op=OP.subtract)
                nc.vector.tensor_tensor(out=ytmp, in0=ytmp,
                                        in1=_bap(rstd, [[0, d]]), op=OP.mult)
                if not pl.mlp_ln_id[l]:
                    gape = gb_sb[:]
                    g3 = bass.AP(tensor=gape.tensor,
                                 offset=gape.offset + (2 * l) * 192,
                                 ap=[gape.ap[0], [0, nt], [1, d]])
                    b3 = bass.AP(tensor=gape.tensor,
                                 offset=gape.offset + (2 * l + 1) * 192,
                                 ap=[gape.ap[0], [0, nt], [1, d]])
                    nc.vector.tensor_tensor(out=ytmp, in0=ytmp, in1=g3,
                                            op=OP.mult)
                    nc.vector.tensor_tensor(out=ytmp, in0=ytmp, in1=b3,
                                            op=OP.add)
                nc.scalar.activation(
                    out=z2_nm[:, t0 * 128:(t0 + nt) * 128],
                    in_=ztmp[:, 512:512 + nt * d], func=AF.Relu)

            def node_phase(l):
                nc.vector.tensor_tensor(out=agg_nm[:], in0=agg_nm[:],
                                        in1=v_nm[:], op=OP.add)
                with tc.tile_pool(name="tpp", bufs=2, space="PSUM") as tpp:
                    for t in range(NT):
                        tp = tpp.tile([64, P], F32, tag="tp")
                        nc.tensor.transpose(
                            tp[:], agg_nm[:, t * 64:(t + 1) * 64], ident_f[:])
                        nc.vector.tensor_copy(
                            out=h1_fm[0:64, t * P:(t + 1) * P], in_=tp[:])
                with tc.tile_pool(name="zpp", bufs=3, space="PSUM") as zpp:
                    for ch in range((NT + 3) // 4):
                        t0, t1 = ch * 4, min(ch * 4 + 4, NT)
                        zps = zpp.tile([P, 512], F32, tag="zps")
                        for t in range(t0, t1):
                            nc.tensor.matmul(
                                zps[:, (t - t0) * 128:(t - t0 + 1) * 128],
                                lhsT=h1_fm[:, t * P:(t + 1) * P],
                                rhs=w1_sb[:, l * 128:(l + 1) * 128],
                                start=True, stop=True)
                        ln_relu_chunk(l, zps, t1 - t0, t0)
                with tc.tile_pool(name="tp2p", bufs=2, space="PSUM") as tp2p:
                    for t in range(NT):
                        tp2 = tp2p.tile([P, P], BF16, tag="tp2")
                        nc.tensor.transpose(
                            tp2[:], z2_nm[:, t * 128:(t + 1) * 128],
                            ident_b[:])
                        nc.vector.tensor_copy(
                            out=z2_fm[:, t * P:(t + 1) * P], in_=tp2[:])
                with tc.tile_pool(name="hpp", bufs=3, space="PSUM") as hpp:
                    for ch in range((NT + 7) // 8):
                        t0, t1 = ch * 8, min(ch * 8 + 8, NT)
                        hps = hpp.tile([P, 512], F32, tag="hps")
                        for t in range(t0, t1):
                            nc.tensor.matmul(
                                hps[:, (t - t0) * 64:(t - t0 + 1) * 64],
                                lhsT=z2_fm[:, t * P:(t + 1) * P],
                                rhs=w2_sb[:, l * 64:(l + 1) * 64],
                                start=True, stop=True)
                        sl = slice(t0 * 64, t1 * 64)
                        psl = hps[:, :(t1 - t0) * 64]
                        if not pl.b2_zero[l]:
                            b2ap = b2_sb[:]
                            nc.vector.tensor_tensor(
                                out=psl, in0=psl,
                                in1=bass.AP(tensor=b2ap.tensor,
                                            offset=b2ap.offset + l * 64,
                                            ap=[b2ap.ap[0], [0, t1 - t0],
                                                [1, 64]]),
                                op=OP.add)
                        if l == 0:
                            nc.scalar.copy(out=h_res[:, sl], in_=psl)
                        else:
                            nc.vector.tensor_tensor(
                                out=h_res[:, sl], in0=h_res[:, sl], in1=psl,
                                op=OP.add)

            def prenorm_relu(l, want_bf):
                d = 64
                z3 = h_res[:].rearrange("p (t d) -> p t d", d=d)
                r1 = ztmp[:, 0:NT]
                nc.vector.tensor_reduce(out=r1, in_=z3, axis=AX.X, op=OP.add)
                nc.scalar.activation(out=sq_sb[:, 0:NT * 64], in_=h_res[:],
                                     func=AF.Square)
                r2 = ztmp[:, 64:64 + NT]
                nc.vector.tensor_reduce(
                    out=r2,
                    in_=sq_sb[:, 0:NT * 64].rearrange("p (t d) -> p t d", d=d),
                    axis=AX.X, op=OP.add)
                mu = ztmp[:, 128:128 + NT]
                nc.vector.tensor_scalar(out=mu, in0=r1, scalar1=1.0 / d,
                                        scalar2=None, op0=OP.mult)
                m2 = ztmp[:, 192:192 + NT]
                nc.vector.tensor_tensor(out=m2, in0=mu, in1=mu, op=OP.mult)
                var = ztmp[:, 256:256 + NT]
                nc.vector.scalar_tensor_tensor(
                    out=var, in0=r2, scalar=1.0 / d, in1=m2,
                    op0=OP.mult, op1=OP.subtract)
                sd = ztmp[:, 320:320 + NT]
                nc.scalar.activation(out=sd, in_=var, func=AF.Sqrt,
                                     bias=eps_sb[:])
                rstd = ztmp[:, 384:384 + NT]
                nc.vector.reciprocal(out=rstd, in_=sd)
                yt = ztmp[:, 512:512 + NT * d].rearrange(
                    "p (t d) -> p t d", d=d)
                nc.vector.tensor_tensor(out=yt, in0=z3, in1=_bap(mu, [[0, d]]),
                                        op=OP.subtract)
                nc.vector.tensor_tensor(out=yt, in0=yt,
                                        in1=_bap(rstd, [[0, d]]), op=OP.mult)
                if not pl.ln_id[l]:
                    gape = gb_sb[:]
                    g3 = bass.AP(tensor=gape.tensor,
                                 offset=gape.offset + (2 * l) * 192 + 128,
                                 ap=[gape.ap[0], [0, NT], [1, d]])
                    b3 = bass.AP(tensor=gape.tensor,
                                 offset=gape.offset + (2 * l + 1) * 192 + 128,
                                 ap=[gape.ap[0], [0, NT], [1, d]])
                    nc.vector.tensor_tensor(out=yt, in0=yt, in1=g3, op=OP.mult)
                    nc.vector.tensor_tensor(out=yt, in0=yt, in1=b3, op=OP.add)
                nc.scalar.activation(out=v_nm[:],
                                     in_=ztmp[:, 512:512 + NT * d],
                                     func=AF.Relu)
                if want_bf:
                    nc.vector.tensor_copy(out=zbf[:], in_=v_nm[:])

            def head_phase():
                prenorm_relu(0, False)
                with tc.tile_pool(name="hd", bufs=1) as hd, \
                     tc.tile_pool(name="hdps", bufs=2, space="PSUM") as hdps:
                    y_fm = hd.tile([65, NT * P], F32, tag="yfm")
                    nc.vector.memset(y_fm[64:65, :], 1.0)
                    for t in range(NT):
                        tp = hdps.tile([64, P], F32, tag="hdtp")
                        nc.tensor.transpose(
                            tp[:], v_nm[:, t * 64:(t + 1) * 64], ident_f[:])
                        nc.vector.tensor_copy(
                            out=y_fm[0:64, t * P:(t + 1) * P], in_=tp[:])
                    ops = hdps.tile([P, NT * COUT], F32, tag="ops")
                    for t in range(NT):
                        nc.tensor.matmul(
                            ops[:, t * COUT:(t + 1) * COUT],
                            lhsT=y_fm[:, t * P:(t + 1) * P],
                            rhs=linw_sb[:], start=True, stop=True)
                    y_sb = hd.tile([P, NT * COUT], F32, tag="ysb")
                    nc.vector.tensor_copy(out=y_sb[:], in_=ops[:])
                    nc.sync.dma_start(
                        out=d_out.rearrange("(t p) d -> p t d", p=P),
                        in_=y_sb[:].rearrange("p (t d) -> p t d", d=COUT))

            enc_phase()
            for l in range(L):
                edge_phase(l)
                node_phase(l)
                if l < L - 1:
                    prenorm_relu(l + 1, True)
                    nc.sync.dma_start(
                        out=cc_in[l + 1][:].rearrange("(t p) d -> p t d", p=P),
                        in_=zbf[:].rearrange("p (t d) -> p t d", d=64))
                    nc.gpsimd.collective_compute(
                        "AllGather", OP.bypass,
                        ins=[cc_in[l + 1][:]], outs=[g_full[l + 1][:]],
                        replica_groups=[list(range(NC))])
            head_phase()

    nc.compile()
    return nc


def in_maps(pl: "Plan"):
    bf = mybir.dt.np(BF16)
    maps = []
    for c in range(pl.NC):
        m = {
            "x_t": pl.x_t[c],
            "src_idx": pl.src_idx[c],
            "eaT": pl.eaT[c].astype(bf),
            "sel": pl.sel[c].astype(bf),
            "enc_w": pl.enc_w,
            "eenc_w": pl.eenc_w.astype(bf),
            "w1": pl.w1,
            "w2": pl.w2.astype(bf),
            "b2": pl.b2,
            "lin_w": pl.lin_w,
        }
        if pl.any_gb:
            gb = np.zeros((2 * pl.L, 192), np.float32)
            for l in range(pl.L):
                gb[2 * l, :128] = pl.mlp_ln_g[l]
                gb[2 * l + 1, :128] = pl.mlp_ln_b[l]
                gb[2 * l, 128:192] = pl.ln_g[l]
                gb[2 * l + 1, 128:192] = pl.ln_b[l]
            m["ln_gb"] = gb
        maps.append(m)
    return maps


_CACHE = {}


def kernel(**inputs) -> np.ndarray:
    from concourse.bass_utils import run_bass_kernel_spmd
    pl = Plan(inputs)
    key = (pl.N, pl.E, pl.KS)
    if key not in _CACHE:
        _CACHE[key] = build(pl)
    nc = _CACHE[key]
    res = run_bass_kernel_spmd(nc, in_maps(pl), core_ids=list(range(pl.NC)))
    out = np.concatenate([res.results[c]["y_out"] for c in range(pl.NC)], 0)
    return np.ascontiguousarray(out[: pl.N])

